# revision 1
# baseline (speedup 1.0000x reference)
"""MoE layer (top-2 routing, E=8 experts) on 8 Trainium2 NeuronCores.

Strategy (expert-parallel, per sharding hint):
 - Host computes the router (softmax over x@Wg+bg, top-2) and dispatches
   each (token, gate) pair to its expert's core: core e gets the tokens
   routed to expert e (gathered, transposed to [D, C], zero-padded to a
   common capacity C).
 - Core e runs a fused MLP kernel for expert e:
       hT = Gelu(W1^T-chunks @ xT + b1)   (PSUM -> SBUF via ACT, bias fused)
       y  = (h @ W2) * gate               (accumulated in PSUM across all
                                           24 F-chunks, gate fused on evict)
   Matmuls run in float32r (full-rate fp32) with fp32 PSUM accumulation.
 - Host scatter-adds the per-expert outputs back into [N, D] and adds the
   (separable) b2 term: sum_k gate_k * b2[e_k].
"""

import numpy as np

B, T, D = 4, 2048, 768
E, F, TOPK = 8, 4 * 768, 2
N = B * T
P = 128
TB = 384          # tokens per on-device block (3 x 128)
NCORES = 8

_nc_cache = {}


def _route(x_flat, Wg, bg):
    """Replicate reference routing: softmax gates, top-2 (ties -> lower idx)."""
    logits = x_flat.astype(np.float64) @ Wg.astype(np.float64) + bg.astype(np.float64)
    logits -= logits.max(axis=-1, keepdims=True)
    eg = np.exp(logits)
    gates = eg / eg.sum(axis=-1, keepdims=True)          # [N, E] f64
    top2 = np.argsort(-gates, axis=-1, kind="stable")[:, :TOPK]   # [N, 2]
    g2 = np.take_along_axis(gates, top2, axis=-1).astype(np.float32)
    return top2, g2


def _build_nc(C, TBo=None, w1q=4, hbufs=3, p1bufs=2, skip_wdma=False):
    import concourse.bacc as bacc
    import concourse.mybir as mybir
    import concourse.tile as tile

    f32 = mybir.dt.float32
    f32r = mybir.dt.float32r
    Gelu = mybir.ActivationFunctionType.Gelu

    KO1 = D // P          # 6 contraction chunks for x@W1
    KO2 = F // P          # 24 contraction chunks for h@W2
    # token blocks: 384s plus 256s so C only needs 128 granularity
    # (psum free dim must stay >= 256 for full-rate fp32r)
    assert C % 128 == 0 and C >= 256
    b384, rem = divmod(C, 384)
    if rem == 0:
        blocks = [384] * b384
    elif rem == 128:
        blocks = [384] * (b384 - 1) + [256, 256]
    else:
        blocks = [384] * b384 + [256]
    assert sum(blocks) == C
    DH = 2                # output D split (psum free <= 512 for f32)
    DHW = D // DH         # 384

    nc = bacc.Bacc("TRN2", target_bir_lowering=False)

    xT = nc.dram_tensor("xT", [D, C], f32r, kind="ExternalInput")
    w1 = nc.dram_tensor("w1", [D, F], f32r, kind="ExternalInput")
    b1 = nc.dram_tensor("b1", [F], f32, kind="ExternalInput")
    w2 = nc.dram_tensor("w2", [F, D], f32r, kind="ExternalInput")
    gates = nc.dram_tensor("gates", [C], f32, kind="ExternalInput")
    y = nc.dram_tensor("y", [C, D], f32, kind="ExternalOutput")

    with tile.TileContext(nc) as tc:
        with (
            tc.tile_pool(name="wpool", bufs=1) as wpool,
            tc.tile_pool(name="xpool", bufs=2) as xpool,
            tc.tile_pool(name="hpool", bufs=hbufs) as hpool,
            tc.tile_pool(name="ypool", bufs=2) as ypool,
            tc.tile_pool(name="psum1", bufs=p1bufs, space="PSUM") as psum1,
            tc.tile_pool(name="psumy", bufs=1, space="PSUM") as psumy,
        ):
            # Small constants first (needed by the first ACT/DVE evicts).
            b1_sb = wpool.tile([P, KO2], f32, tag="b1")
            nc.sync.dma_start(b1_sb[:], b1[:].rearrange("(fo p) -> p fo", p=P))
            gates_sb = wpool.tile([P, C // P], f32, tag="gates")
            nc.sync.dma_start(gates_sb[:], gates[:].rearrange("(mo p) -> p mo", p=P))
            # Resident weights, split per chunk and DMA'd in first-block
            # consumption order (w1 quarter q feeds fc in [q*6, q*6+6), then
            # w2[fc] for those fc) so the PE can start ~4 MB in instead of
            # waiting for the full 19 MB weight load.
            FQ = F // w1q
            w1_ap = w1[:, :].rearrange("(ko p) f -> ko p f", p=P)
            w2_ap = w2[:, :].rearrange("(ko p) d -> ko p d", p=P)
            w1_sb = [[None] * w1q for _ in range(KO1)]
            w2_sb = [None] * KO2
            for q in range(w1q):
                for kc in range(KO1):
                    t = wpool.tile([P, FQ], f32r, tag=f"w1_{kc}_{q}",
                                   name=f"w1sb_{kc}_{q}")
                    if not skip_wdma:
                        nc.sync.dma_start(t[:], w1_ap[kc][:, q * FQ:(q + 1) * FQ])
                    w1_sb[kc][q] = t
                for fc in range(q * (KO2 // w1q), (q + 1) * (KO2 // w1q)):
                    t = wpool.tile([P, D], f32r, tag=f"w2_{fc}", name=f"w2sb_{fc}")
                    if not skip_wdma:
                        nc.sync.dma_start(t[:], w2_ap[fc])
                    w2_sb[fc] = t

            xT_ap = xT[:, :].rearrange("(ko p) c -> p ko c", p=P)

            tok0 = 0
            for tb, TBl in enumerate(blocks):
                TS = TBl // P
                x_sb = xpool.tile([P, KO1, TBl], f32r, tag="x")
                nc.gpsimd.dma_start(x_sb[:], xT_ap[:, :, tok0:tok0 + TBl])

                ypsum = [
                    [
                        psumy.tile([P, DHW], f32, tag=f"y_{ts}_{dh}",
                                   name=f"ypsum_{ts}_{dh}")
                        for dh in range(DH)
                    ]
                    for ts in range(TS)
                ]
                # Software-pipeline: emit mm1 group LA chunks ahead of the
                # mm2 stream so the PE has fill work while the previous
                # block's y-psum banks drain (avoids in-order head-of-line
                # blocking at block boundaries).
                LA = 2
                h_q = {}
                for i in range(KO2 + LA):
                    if i < KO2:
                        fc = i
                        p1 = psum1.tile([P, TBl], f32, tag="p1")
                        for kc in range(KO1):
                            nc.tensor.matmul(
                                p1[:],
                                lhsT=w1_sb[kc][fc // (KO2 // w1q)][
                                    :, (fc % (KO2 // w1q)) * P:
                                    (fc % (KO2 // w1q) + 1) * P],
                                rhs=x_sb[:, kc, :],
                                start=(kc == 0),
                                stop=(kc == KO1 - 1),
                            )
                        h_sb = hpool.tile([P, TBl], f32r, tag="h")
                        nc.scalar.activation(h_sb[:], p1[:], Gelu,
                                             bias=b1_sb[:, fc:fc + 1])
                        h_q[fc] = h_sb
                    j = i - LA
                    if j >= 0:
                        h_j = h_q.pop(j)
                        for ts in range(TS):
                            for dh in range(DH):
                                nc.tensor.matmul(
                                    ypsum[ts][dh][:],
                                    lhsT=h_j[:, ts * P:(ts + 1) * P],
                                    rhs=w2_sb[j][:, dh * DHW:(dh + 1) * DHW],
                                    start=(j == 0),
                                    stop=(j == KO2 - 1),
                                )
                y_sb = ypool.tile([P, TS, D], f32, tag="y")
                mo0 = tok0 // P
                for ts in range(TS):
                    g_ap = gates_sb[:, mo0 + ts: mo0 + ts + 1]
                    for dh in range(DH):
                        nc.vector.tensor_scalar_mul(
                            y_sb[:, ts, dh * DHW:(dh + 1) * DHW],
                            ypsum[ts][dh][:],
                            g_ap,
                        )
                nc.gpsimd.dma_start(
                    y[tok0:tok0 + TBl, :].rearrange("(ts p) d -> p ts d", p=P),
                    y_sb[:],
                )
                tok0 += TBl
    nc.compile()
    return nc


def kernel(x, Wg, bg, W1, b1, W2, b2):
    from concourse.bass_utils import run_bass_kernel_spmd

    x = np.asarray(x, dtype=np.float32)
    Wg = np.asarray(Wg, dtype=np.float32)
    bg = np.asarray(bg, dtype=np.float32)
    W1 = np.asarray(W1, dtype=np.float32)
    b1 = np.asarray(b1, dtype=np.float32)
    W2 = np.asarray(W2, dtype=np.float32)
    b2 = np.asarray(b2, dtype=np.float32)

    x_flat = x.reshape(-1, D)
    top2, g2 = _route(x_flat, Wg, bg)

    # Dispatch: token lists per expert
    idx_e = []
    gate_e = []
    for e in range(E):
        sel = np.nonzero(top2 == e)
        idx_e.append(sel[0].astype(np.int64))                  # token ids
        gate_e.append(g2[sel[0], sel[1]].astype(np.float32))   # their gates
    counts = [len(i) for i in idx_e]
    C = max(max(counts), 129)
    C = ((C + P - 1) // P) * P

    if C not in _nc_cache:
        _nc_cache[C] = _build_nc(C)
    nc = _nc_cache[C]

    in_maps = []
    for e in range(E):
        n_e = counts[e]
        xTe = np.zeros((D, C), dtype=np.float32)
        if n_e:
            xTe[:, :n_e] = x_flat[idx_e[e]].T
        ge = np.zeros((C,), dtype=np.float32)
        ge[:n_e] = gate_e[e]
        in_maps.append({
            "xT": xTe,
            "w1": np.ascontiguousarray(W1[e]),
            "b1": np.ascontiguousarray(b1[e]),
            "w2": np.ascontiguousarray(W2[e]),
            "gates": ge,
        })

    res = run_bass_kernel_spmd(nc, in_maps, core_ids=list(range(NCORES)))

    out = np.zeros((N, D), dtype=np.float32)
    for e in range(E):
        n_e = counts[e]
        if n_e:
            out[idx_e[e]] += res.results[e]["y"][:n_e]
    # separable b2 term: sum_k gate_k * b2[e_k]
    if np.any(b2):
        out += g2[:, 0:1] * b2[top2[:, 0]] + g2[:, 1:2] * b2[top2[:, 1]]
    return out.reshape(B, T, D)



# revision 2
# speedup vs baseline: 1.3005x; 1.3005x over previous
"""MoE layer (top-2 routing, E=8 experts) on 8 Trainium2 NeuronCores.

Strategy (expert-parallel, per sharding hint):
 - Host computes the router (softmax over x@Wg+bg, top-2) and dispatches
   each (token, gate) pair to its expert's core: core e gets the tokens
   routed to expert e (gathered, transposed, zero-padded to a common
   capacity C).
 - Matmuls run as fp8e4 (e4m3) DoubleRow-mode matmuls (K=256 per pass,
   0.5 cycles/row -> 2x the fp32r/bf16 rate).  Full fp32-like accuracy is
   recovered with a hi/lo residual decomposition of both operands and a
   3-term product accumulated in the same fp32 PSUM group:
       a @ b ~= a_hi@b_hi + a_lo@b_hi + a_hi@b_lo      (drops only lo*lo)
   where v_hi = fp8(v), v_lo = fp8(v - v_hi)  (7+ effective mantissa bits).
 - Core e computes for expert e:
       h  = Gelu((1/S1) * [3-term fp8 x@W1 psum] + b1)   (ACT, f32)
       hh = fp8(SH * h)                                  (Pool)
       hl = fp8(SH * h - hh)                             (DVE)
       y  = [3-term fp8 h@W2 psum] * gates'              (DVE evict;
            gates' = gate/(SH*SW2) folds away all the quant scales)
 - Host scatter-adds the per-expert outputs back into [N, D] and adds the
   (separable) b2 term: sum_k gate_k * b2[e_k].
"""

import numpy as np
import ml_dtypes

B, T, D = 4, 2048, 768
E, F, TOPK = 8, 4 * 768, 2
N = B * T
P = 128
NCORES = 8

E4 = ml_dtypes.float8_e4m3
SX = 32.0       # x scale before fp8 (max |x| ~ 5.1 -> 164 < 240)
SW1 = 1024.0    # W1 scale (max ~0.11 -> 111)
SH = 32.0       # h scale  (max ~3.9 -> 125)
SW2 = 1024.0    # W2 scale

_nc_cache = {}


def _route(x_flat, Wg, bg):
    """Replicate reference routing: softmax gates, top-2 (ties -> lower idx)."""
    logits = x_flat.astype(np.float64) @ Wg.astype(np.float64) + bg.astype(np.float64)
    logits -= logits.max(axis=-1, keepdims=True)
    eg = np.exp(logits)
    gates = eg / eg.sum(axis=-1, keepdims=True)          # [N, E] f64
    top2 = np.argsort(-gates, axis=-1, kind="stable")[:, :TOPK]   # [N, 2]
    g2 = np.take_along_axis(gates, top2, axis=-1).astype(np.float32)
    return top2, g2


def _hilo_pack(a, s):
    """a: [K, M] with contraction along rows.  Scale by s, split into fp8
    hi/lo, pack each as [128, K//256, 2, M] (partition, double-tile,
    k-tile, col) matching the DoubleRow SBUF layout."""
    sc = a * np.float32(s)
    hi = sc.astype(E4)
    lo = (sc - hi.astype(np.float32)).astype(E4)

    def pack(v):
        nkd = v.shape[0] // 256
        return np.ascontiguousarray(
            v.reshape(nkd, 2, P, v.shape[1]).transpose(2, 0, 1, 3))

    return pack(hi), pack(lo)


def _build_nc(C, LA=3):
    import concourse.bacc as bacc
    import concourse.mybir as mybir
    import concourse.tile as tile

    f32 = mybir.dt.float32
    fp8 = mybir.dt.float8e4
    Gelu = mybir.ActivationFunctionType.Gelu
    DR = mybir.MatmulPerfMode.DoubleRow
    MUL = mybir.AluOpType.mult
    SUB = mybir.AluOpType.subtract

    KO2 = F // P          # 24 h chunks
    ND1 = D // 256        # 3 double-k-tiles for x@W1
    ND2 = F // 256        # 12 double-k-tiles for h@W2
    NQ = 4                # weight DMA staggered in 4 quarters
    FQ = F // NQ          # 768
    DH = 2                # output D split (psum tile free dim 384)
    DHW = D // DH
    inv_S1 = 1.0 / (SX * SW1)

    assert C % 128 == 0 and C >= 256
    b384, rem = divmod(C, 384)
    if rem == 0:
        blocks = [384] * b384
    elif rem == 128:
        blocks = [384] * (b384 - 1) + [256, 256]
    else:
        blocks = [384] * b384 + [256]
    assert sum(blocks) == C

    nc = bacc.Bacc("TRN2", target_bir_lowering=False)

    xh = nc.dram_tensor("xh", [P, ND1, 2, C], fp8, kind="ExternalInput")
    xl = nc.dram_tensor("xl", [P, ND1, 2, C], fp8, kind="ExternalInput")
    w1h = nc.dram_tensor("w1h", [P, ND1, 2, F], fp8, kind="ExternalInput")
    w1l = nc.dram_tensor("w1l", [P, ND1, 2, F], fp8, kind="ExternalInput")
    w2h = nc.dram_tensor("w2h", [P, ND2, 2, D], fp8, kind="ExternalInput")
    w2l = nc.dram_tensor("w2l", [P, ND2, 2, D], fp8, kind="ExternalInput")
    b1 = nc.dram_tensor("b1", [F], f32, kind="ExternalInput")
    gates = nc.dram_tensor("gates", [C], f32, kind="ExternalInput")
    y = nc.dram_tensor("y", [C, D], f32, kind="ExternalOutput")

    with tile.TileContext(nc) as tc:
        with (
            tc.tile_pool(name="wpool", bufs=1) as wpool,
            tc.tile_pool(name="xpool", bufs=2) as xpool,
            tc.tile_pool(name="hfpool", bufs=4) as hfpool,
            tc.tile_pool(name="hpool", bufs=2) as hpool,
            tc.tile_pool(name="ypool", bufs=2) as ypool,
            tc.tile_pool(name="psum1", bufs=2, space="PSUM") as psum1,
            tc.tile_pool(name="psumy", bufs=1, space="PSUM") as psumy,
        ):
            # Small constants first (needed by the first ACT/DVE evicts).
            b1_sb = wpool.tile([P, KO2], f32, tag="b1", name="b1_sb")
            nc.sync.dma_start(b1_sb[:], b1[:].rearrange("(fo p) -> p fo", p=P))
            gates_sb = wpool.tile([P, C // P], f32, tag="gates", name="gates_sb")
            nc.sync.dma_start(gates_sb[:], gates[:].rearrange("(mo p) -> p mo", p=P))

            # Resident weights, DMA'd in first-block consumption order so the
            # PE can start ~2.3 KB/partition in instead of after the full
            # 73.7 KB/partition weight load.
            w1h_t, w1l_t, w2h_t, w2l_t = [], [], [], []
            for q in range(NQ):
                t = wpool.tile([P, ND1, 2, FQ], fp8, tag=f"w1h_{q}",
                               name=f"w1h_sb{q}")
                nc.sync.dma_start(t[:], w1h[:, :, :, q * FQ:(q + 1) * FQ])
                w1h_t.append(t)
                t = wpool.tile([P, ND1, 2, FQ], fp8, tag=f"w1l_{q}",
                               name=f"w1l_sb{q}")
                nc.sync.dma_start(t[:], w1l[:, :, :, q * FQ:(q + 1) * FQ])
                w1l_t.append(t)
                nk = ND2 // NQ
                t = wpool.tile([P, nk, 2, D], fp8, tag=f"w2h_{q}",
                               name=f"w2h_sb{q}")
                nc.sync.dma_start(t[:], w2h[:, q * nk:(q + 1) * nk, :, :])
                w2h_t.append(t)
                t = wpool.tile([P, nk, 2, D], fp8, tag=f"w2l_{q}",
                               name=f"w2l_sb{q}")
                nc.sync.dma_start(t[:], w2l[:, q * nk:(q + 1) * nk, :, :])
                w2l_t.append(t)

            tok0 = 0
            for tb, TBl in enumerate(blocks):
                TS = TBl // P
                mo0 = tok0 // P
                xh_sb = xpool.tile([P, ND1, 2, TBl], fp8, tag="xh", name="xh_sb")
                nc.gpsimd.dma_start(xh_sb[:], xh[:, :, :, tok0:tok0 + TBl])
                xl_sb = xpool.tile([P, ND1, 2, TBl], fp8, tag="xl", name="xl_sb")
                nc.gpsimd.dma_start(xl_sb[:], xl[:, :, :, tok0:tok0 + TBl])

                # h chunk-pair tiles: contiguous [P, 2, TBl] so a DoubleRow
                # lhsT AP can span both k-tiles of a 256-deep contraction.
                hh_sb = [hpool.tile([P, 2, TBl], fp8, tag=f"hh_{k}",
                                    name=f"hh_sb{k}") for k in range(ND2)]
                hl_sb = [hpool.tile([P, 2, TBl], fp8, tag=f"hl_{k}",
                                    name=f"hl_sb{k}") for k in range(ND2)]
                ypsum = [
                    [psumy.tile([P, DHW], f32, tag=f"y_{ts}_{dh}",
                                name=f"ypsum_{ts}_{dh}") for dh in range(DH)]
                    for ts in range(TS)
                ]
                y_sb = ypool.tile([P, TS, D], f32, tag="y", name="y_sb")

                # Software pipeline: mm1 chunk i runs LA chunks ahead of the
                # mm2 stream (mm2 consumes h chunks in pairs).
                for i in range(KO2 + LA):
                    if i < KO2:
                        q, fq = i // (KO2 // NQ), i % (KO2 // NQ)
                        p1 = psum1.tile([P, TBl], f32, tag="p1", name="p1_sb")
                        terms = ((xh_sb, w1h_t[q]), (xl_sb, w1h_t[q]),
                                 (xh_sb, w1l_t[q]))
                        for t, (xa, wa) in enumerate(terms):
                            for kd in range(ND1):
                                nc.tensor.matmul(
                                    p1[:],
                                    lhsT=wa[:, kd, :, fq * P:(fq + 1) * P],
                                    rhs=xa[:, kd, :, :],
                                    start=(t == 0 and kd == 0),
                                    stop=(t == 2 and kd == ND1 - 1),
                                    perf_mode=DR,
                                )
                        hf = hfpool.tile([P, TBl], f32, tag="hf", name="hf_sb")
                        nc.scalar.activation(hf[:], p1[:], Gelu,
                                             bias=b1_sb[:, i:i + 1],
                                             scale=inv_S1)
                        kp, tp = i // 2, i % 2
                        nc.gpsimd.tensor_scalar_mul(hh_sb[kp][:, tp, :],
                                                    hf[:], SH)
                        nc.vector.scalar_tensor_tensor(hl_sb[kp][:, tp, :],
                                                       hf[:], SH,
                                                       hh_sb[kp][:, tp, :],
                                                       MUL, SUB)
                    j = i - LA
                    if j >= 0 and j % 2 == 1:
                        kd = j // 2
                        q = kd // (ND2 // NQ)
                        kq = kd % (ND2 // NQ)
                        terms2 = ((hh_sb[kd], w2h_t[q]), (hl_sb[kd], w2h_t[q]),
                                  (hh_sb[kd], w2l_t[q]))
                        for ts in range(TS):
                            for t, (ha, wa) in enumerate(terms2):
                                for dh in range(DH):
                                    nc.tensor.matmul(
                                        ypsum[ts][dh][:],
                                        lhsT=ha[:, :, ts * P:(ts + 1) * P],
                                        rhs=wa[:, kq, :, dh * DHW:(dh + 1) * DHW],
                                        start=(kd == 0 and t == 0),
                                        stop=(kd == ND2 - 1 and t == 2),
                                        perf_mode=DR,
                                    )
                        if kd == ND2 - 1:
                            for ts in range(TS):
                                g_ap = gates_sb[:, mo0 + ts: mo0 + ts + 1]
                                for dh in range(DH):
                                    nc.vector.tensor_scalar_mul(
                                        y_sb[:, ts, dh * DHW:(dh + 1) * DHW],
                                        ypsum[ts][dh][:],
                                        g_ap,
                                    )
                                nc.gpsimd.dma_start(
                                    y[tok0 + ts * P:tok0 + (ts + 1) * P, :],
                                    y_sb[:, ts, :],
                                )
                tok0 += TBl
    nc.compile()
    return nc


def kernel(x, Wg, bg, W1, b1, W2, b2):
    from concourse.bass_utils import run_bass_kernel_spmd

    x = np.asarray(x, dtype=np.float32)
    Wg = np.asarray(Wg, dtype=np.float32)
    bg = np.asarray(bg, dtype=np.float32)
    W1 = np.asarray(W1, dtype=np.float32)
    b1 = np.asarray(b1, dtype=np.float32)
    W2 = np.asarray(W2, dtype=np.float32)
    b2 = np.asarray(b2, dtype=np.float32)

    x_flat = x.reshape(-1, D)
    top2, g2 = _route(x_flat, Wg, bg)

    # Dispatch: token lists per expert
    idx_e = []
    gate_e = []
    for e in range(E):
        sel = np.nonzero(top2 == e)
        idx_e.append(sel[0].astype(np.int64))                  # token ids
        gate_e.append(g2[sel[0], sel[1]].astype(np.float32))   # their gates
    counts = [len(i) for i in idx_e]
    C = max(max(counts), 256)
    C = ((C + P - 1) // P) * P

    if C not in _nc_cache:
        _nc_cache[C] = _build_nc(C)
    nc = _nc_cache[C]

    in_maps = []
    for e in range(E):
        n_e = counts[e]
        xTe = np.zeros((D, C), dtype=np.float32)
        if n_e:
            xTe[:, :n_e] = x_flat[idx_e[e]].T
        xh_p, xl_p = _hilo_pack(xTe, SX)
        w1h_p, w1l_p = _hilo_pack(W1[e], SW1)
        w2h_p, w2l_p = _hilo_pack(W2[e], SW2)
        ge = np.zeros((C,), dtype=np.float32)
        ge[:n_e] = gate_e[e] / np.float32(SH * SW2)
        in_maps.append({
            "xh": xh_p, "xl": xl_p,
            "w1h": w1h_p, "w1l": w1l_p,
            "w2h": w2h_p, "w2l": w2l_p,
            "b1": np.ascontiguousarray(b1[e]),
            "gates": ge,
        })

    res = run_bass_kernel_spmd(nc, in_maps, core_ids=list(range(NCORES)))

    out = np.zeros((N, D), dtype=np.float32)
    for e in range(E):
        n_e = counts[e]
        if n_e:
            out[idx_e[e]] += res.results[e]["y"][:n_e]
    # separable b2 term: sum_k gate_k * b2[e_k]
    if np.any(b2):
        out += g2[:, 0:1] * b2[top2[:, 0]] + g2[:, 1:2] * b2[top2[:, 1]]
    return out.reshape(B, T, D)
